# revision 1
# baseline (speedup 1.0000x reference)
"""AdaptiveChannelAttention Trainium2 kernel.

Batch-parallel over 8 NeuronCores (one sample per core, weights replicated).
Per-core program (C=256 channels, N=4096 tokens, 8 heads x 32 head-dim):

  qk   = x^T @ Wqk              (token layout, PE, bf16)
  S    = q^T @ k                (per-head 32x32 blocks of a 256x256 gram)
  l2   : column sums of q^2/k^2 via ones-matmul, rsqrt via ln->exp on ACT
  A    = exp(S * rq * rk * tau) / rowsum   (no max-sub: |S*rq*rk| <= tau)
  v    = Wv^T @ x               (channel layout, PE)
  conv = 3x3 depthwise on v: all 9 taps as per-channel diagonal matmuls
         on PE accumulated in PSUM; then BN+GELU fused into one ACT op.
         Interleaved per chunk-pair with the spatial gate below.
  sm   = spatial gate: conv @ si_w1 -> BN+GELU -> @ si_w2 broadcast to 128
         partitions via a replicated stationary; sigmoid via tanh.
  cm   = channel gate from GAP(att), computed algebraically as
         invr * (A_exp @ rowsum(v)) so it never waits on the attention output
  out  = Wproj^T @ (att*sig(sm) + conv*sig(cm)) + b   (channel layout = NCHW)

All matmuls bf16 (fp32 is 4x slower on PE); PSUM accumulation fp32.
BN constants, 1/4096 GAP factor and branch scales are folded on the host.
"""

import sys
import numpy as np

sys.path.insert(0, "/opt/trn_rl_repo")

import ml_dtypes  # noqa: E402
import concourse.bass as bass  # noqa: E402
from concourse import bacc  # noqa: E402
import concourse.tile as tile  # noqa: E402
from concourse import mybir  # noqa: E402
from concourse.bass_utils import run_bass_kernel_spmd  # noqa: E402

BF16 = mybir.dt.bfloat16
F32 = mybir.dt.float32
AF = mybir.ActivationFunctionType
ALU = mybir.AluOpType
NPBF16 = ml_dtypes.bfloat16

EPS_BN = 1e-3
C = 256
N = 4096
HW_SIDE = 64
PADW = 66  # 64 + zero border
NCORES = 8
NT = 32          # token tiles of 128
NF = 8           # free chunks of 512
TAPS = [(dy, dx) for dy in (-1, 0, 1) for dx in (-1, 0, 1)]
PE_TAPS = tuple(range(9))   # all taps as diagonal matmuls on PE
ID_SLOT = 9                 # identity matrix slot in the diag-const pack


def _bf(a):
    return np.ascontiguousarray(np.asarray(a, np.float32)).astype(NPBF16)


def _f32(a):
    return np.ascontiguousarray(np.asarray(a, np.float32))


def _build_consts(p):
    """Host-side folding of all weights/BN constants. Returns name->np array."""
    c = {}
    wqkv = np.asarray(p["w_qkv"], np.float32)          # (256, 768)
    c["cwqk"] = _bf(wqkv[:, :512].reshape(2, 128, 512))
    c["cwv"] = _bf(wqkv[:, 512:].reshape(2, 128, 256))
    c["cwproj"] = _bf(np.asarray(p["w_proj"], np.float32).reshape(2, 128, 256))
    c["cbproj"] = _f32(np.asarray(p["b_proj"]).reshape(2, 128, 1))

    # conv BN fold: y = conv*s + t
    s_dw = p["dw_gamma"] / np.sqrt(p["dw_var"] + EPS_BN)
    t_dw = (p["dw_b"] - p["dw_mean"]) * s_dw + p["dw_beta"]
    c["csdw"] = _f32(s_dw.reshape(2, 128, 1))
    c["ctdw"] = _f32(t_dw.reshape(2, 128, 1))

    # depthwise taps: per-channel scalars + diagonal matrices (+ identity)
    dwk = np.asarray(p["dw_k"], np.float32).reshape(C, 9)  # (c, 3*ky+kx)
    c["cktap"] = _f32(dwk.reshape(2, 128, 9))
    diag = np.zeros((2, 128, 10, 128), np.float32)
    for m in range(2):
        for t in range(9):
            np.fill_diagonal(diag[m, :, t, :], dwk[m * 128:(m + 1) * 128, t])
        np.fill_diagonal(diag[m, :, ID_SLOT, :], 1.0)
    c["cdw"] = _bf(diag.reshape(2, 128, 10 * 128))

    # spatial gate: fold BN scale into si_w1
    s_si = p["si_gamma"] / np.sqrt(p["si_var"] + EPS_BN)
    t_si = (p["si_b1"] - p["si_mean"]) * s_si + p["si_beta"]
    c["csiw1"] = _bf((np.asarray(p["si_w1"], np.float32) * s_si[None, :])
                     .reshape(2, 128, 16))
    c["ctsi"] = _f32(t_si.reshape(16, 1))
    c["csiw2"] = _bf(np.tile(np.asarray(p["si_w2"], np.float32), (1, 128)))

    # channel gate: fold BN scale and the 1/4096 GAP mean into ci_w1
    s_ci = p["ci_gamma"] / np.sqrt(p["ci_var"] + EPS_BN)
    t_ci = (p["ci_b1"] - p["ci_mean"]) * s_ci + p["ci_beta"]
    c["cciw1"] = _bf((np.asarray(p["ci_w1"], np.float32) * s_ci[None, :] / N)
                     .reshape(2, 128, 32))
    c["ctci"] = _f32(t_ci.reshape(32, 1))
    c["cciw2"] = _bf(np.asarray(p["ci_w2"], np.float32))           # (32, 256)
    c["ccib2"] = _f32(0.5 * np.asarray(p["ci_b2"]).reshape(2, 128, 1))

    # temperature per k-column (head-major channel order)
    temp = np.asarray(p["temperature"], np.float32).reshape(8)
    c["ctemp"] = _f32(np.repeat(temp, 32).reshape(1, 256))
    return c


def _build_program(si_b2_half, reps=1):
    nc = bacc.Bacc(None)
    x_d = nc.declare_dram_parameter("x", [C, N], F32, isOutput=False)
    cd = {}
    cspecs = {
        "cwqk": ([2, 128, 512], BF16), "cwv": ([2, 128, 256], BF16),
        "cwproj": ([2, 128, 256], BF16), "cbproj": ([2, 128, 1], F32),
        "csdw": ([2, 128, 1], F32), "ctdw": ([2, 128, 1], F32),
        "cktap": ([2, 128, 9], F32), "cdw": ([2, 128, 1280], BF16),
        "csiw1": ([2, 128, 16], BF16), "ctsi": ([16, 1], F32),
        "csiw2": ([16, 128], BF16), "cciw1": ([2, 128, 32], BF16),
        "ctci": ([32, 1], F32), "cciw2": ([32, 256], BF16),
        "ccib2": ([2, 128, 1], F32), "ctemp": ([1, 256], F32),
    }
    for k, (shp, dt) in cspecs.items():
        cd[k] = nc.declare_dram_parameter(k, shp, dt, isOutput=False)
    out_d = nc.declare_dram_parameter("out", [C, N], F32, isOutput=True)

    with tile.TileContext(nc) as tc:
        with (
            tc.tile_pool(name="consts", bufs=1) as pc,
            tc.tile_pool(name="xin", bufs=3) as pxin,
            tc.tile_pool(name="xbf", bufs=1) as pxbf,
            tc.tile_pool(name="qk", bufs=1) as pqk,
            tc.tile_pool(name="sq", bufs=2) as psq,
            tc.tile_pool(name="vp", bufs=1) as pvp,
            tc.tile_pool(name="mid", bufs=1) as pmid,
            tc.tile_pool(name="small", bufs=1) as psm,
            tc.tile_pool(name="outst", bufs=3) as pout,
            tc.tile_pool(name="ps", bufs=2, space="PSUM") as ppb,
            tc.tile_pool(name="pss", bufs=1, space="PSUM") as pps,
            tc.tile_pool(name="pbc", bufs=1, space="PSUM") as pbc,
            tc.tile_pool(name="pst", bufs=1, space="PSUM") as ppt,
        ):
            # ---- constants to SBUF ----
            def cload(name, shape, dt, src):
                t = pc.tile(shape, dt, tag=name, name=name)
                nc.gpsimd.dma_start(t[:], src)
                return t

            wqk = [cload(f"wqk{k}", [128, 512], BF16, cd["cwqk"][k]) for k in range(2)]
            wv = [cload(f"wv{k}", [128, 256], BF16, cd["cwv"][k]) for k in range(2)]
            wproj = [cload(f"wpr{k}", [128, 256], BF16, cd["cwproj"][k]) for k in range(2)]
            bproj = [cload(f"bpr{m}", [128, 1], F32, cd["cbproj"][m]) for m in range(2)]
            sdw = [cload(f"sdw{m}", [128, 1], F32, cd["csdw"][m]) for m in range(2)]
            tdw = [cload(f"tdw{m}", [128, 1], F32, cd["ctdw"][m]) for m in range(2)]
            ktap = [cload(f"ktap{m}", [128, 9], F32, cd["cktap"][m]) for m in range(2)]
            dwdg = [cload(f"dwdg{m}", [128, 1280], BF16, cd["cdw"][m]) for m in range(2)]
            siw1 = [cload(f"siw1{m}", [128, 16], BF16, cd["csiw1"][m]) for m in range(2)]
            tsi = cload("tsi", [16, 1], F32, cd["ctsi"][:])
            siw2 = cload("siw2", [16, 128], BF16, cd["csiw2"][:])
            ciw1 = [cload(f"ciw1{m}", [128, 32], BF16, cd["cciw1"][m]) for m in range(2)]
            tci = cload("tci", [32, 1], F32, cd["ctci"][:])
            ciw2 = cload("ciw2", [32, 256], BF16, cd["cciw2"][:])
            cib2 = [cload(f"cib2{m}", [128, 1], F32, cd["ccib2"][m]) for m in range(2)]
            ctemp = cload("ctemp", [1, 256], F32, cd["ctemp"][:])

            ones_c = pc.tile([128, 1], BF16, tag="ones_c", name="ones_c")
            nc.vector.memset(ones_c[:], 1.0)
            ones_r = pc.tile([1, 128], BF16, tag="ones_r", name="ones_r")
            nc.vector.memset(ones_r[:], 1.0)
            epsv = pc.tile([128, 1], F32, tag="epsv", name="epsv")
            nc.vector.memset(epsv[:], 1e-12)
            sib2v = pc.tile([128, 1], F32, tag="sib2v", name="sib2v")
            nc.vector.memset(sib2v[:], float(si_b2_half))

            for _rep in range(reps):
                # ---- x load + bf16 convert (gpsimd keeps DVE/ACT free) ----
                xbf = [pxbf.tile([128, N], BF16, tag=f"xbf{m}", name=f"xbf{m}")
                       for m in range(2)]
                for hh in range(4):
                    for m in range(2):
                        xf = pxin.tile([128, 1024], F32, tag="xf32", name="xf32")
                        nc.sync.dma_start(
                            xf[:], x_d[m * 128:(m + 1) * 128,
                                       hh * 1024:(hh + 1) * 1024])
                        nc.scalar.activation(
                            xbf[m][:, hh * 1024:(hh + 1) * 1024], xf[:],
                            AF.Copy)

                # ---- qk matmuls (token layout) + copies + squares + ssq ----
                qk_sb = pqk.tile([128, NT * 512], BF16, tag="qk", name="qk")
                ssq_ps = ppt.tile([1, 512], F32, tag="ssq", name="ssq", bufs=1)
                for g in range(4):  # groups of 8 token-tiles
                    sqt = psq.tile([128, 8 * 512], BF16, tag="sq", name="sq")
                    for pair in range(4):
                        nt0 = g * 8 + pair * 2
                        qkp = ppb.tile([128, 1024], F32, tag="ps", name="ps")
                        for half in range(2):
                            nt = nt0 + half
                            for kk in range(2):
                                nc.tensor.matmul(
                                    qkp[:, half * 512:(half + 1) * 512],
                                    xbf[kk][:, nt * 128:(nt + 1) * 128],
                                    wqk[kk][:], start=(kk == 0), stop=(kk == 1))
                        dst = qk_sb[:, nt0 * 512:(nt0 + 2) * 512]
                        if pair % 2 == 0:
                            nc.scalar.activation(dst, qkp[:], AF.Copy)
                        else:
                            nc.vector.tensor_copy(dst, qkp[:])
                    # squares of the whole [q|k] group (one op, FD 4096)
                    qv = qk_sb[:].rearrange("p (t x) -> p t x", x=512)
                    nc.vector.tensor_mul(
                        sqt[:].rearrange("p (t x) -> p t x", x=512),
                        qv[:, g * 8:(g + 1) * 8, :], qv[:, g * 8:(g + 1) * 8, :])
                    for h in range(8):  # ssq accumulation: [sum q | sum k]
                        nc.tensor.matmul(
                            ssq_ps[:], ones_c[:],
                            sqt[:, h * 512:(h + 1) * 512],
                            start=(g == 0 and h == 0), stop=(g == 3 and h == 7))

                # ---- v (channel layout) into zero-padded conv buffer ----
                vpad = [pvp.tile([128, PADW * PADW], BF16, tag=f"vp{m}",
                                 name=f"vp{m}") for m in range(2)]
                for m in range(2):
                    # zero only the 1-px border; interior is fully overwritten
                    rows3 = vpad[m][:].rearrange("p (r c) -> p r c", c=PADW)
                    nc.gpsimd.memset(rows3[:, 0:66:65, :], 0.0)
                    nc.gpsimd.memset(rows3[:, 1:65, 0:66:65], 0.0)
                vp3 = [vpad[m][:].rearrange("p (r c) -> p r c", c=PADW)
                       for m in range(2)]
                vsump = [psm.tile([128, 4], F32, tag=f"vsp{m}", name=f"vsp{m}")
                         for m in range(2)]
                for fp in range(4):  # pairs of 512-chunks
                    for m in range(2):
                        v_ps = ppb.tile([128, 1024], F32, tag="ps", name="ps")
                        for half in range(2):
                            f = fp * 2 + half
                            for kk in range(2):
                                nc.tensor.matmul(
                                    v_ps[:, half * 512:(half + 1) * 512],
                                    wv[kk][:, m * 128:(m + 1) * 128],
                                    xbf[kk][:, f * 512:(f + 1) * 512],
                                    start=(kk == 0), stop=(kk == 1))
                        dst = vp3[m][:, fp * 16 + 1: fp * 16 + 17, 1:65]
                        src = v_ps[:].rearrange("p (r c) -> p r c", c=64)
                        if fp % 2 == 0:
                            nc.scalar.activation(
                                dst, src, AF.Copy,
                                accum_out=vsump[m][:, fp:fp + 1])
                        else:
                            nc.vector.tensor_scalar(
                                dst, src, 1.0, 0.0, ALU.mult, ALU.add,
                                accum_out=vsump[m][:, fp:fp + 1])

                # ---- rq / rk rows ----
                ssum = psm.tile([1, 512], F32, tag="ssum", name="ssum")
                nc.vector.tensor_copy(ssum[:], ssq_ps[:])
                lnr = psm.tile([1, 512], F32, tag="lnr", name="lnr")
                nc.scalar.activation(lnr[:], ssum[:], AF.Ln, bias=epsv[0:1, :])
                rrow_f = psm.tile([1, 512], F32, tag="rrow_f", name="rrow_f")
                nc.scalar.activation(rrow_f[:], lnr[:], AF.Exp, scale=-0.5)
                rrow = psm.tile([1, 512], BF16, tag="rrow", name="rrow")
                nc.vector.tensor_copy(rrow[:, 0:256], rrow_f[:, 0:256])
                nc.vector.tensor_mul(rrow[:, 256:512], rrow_f[:, 256:512],
                                     ctemp[:])

                # rq as per-partition column via K=1 transpose-matmul
                rq_col = [psm.tile([128, 1], F32, tag=f"rq{m}", name=f"rq{m}")
                          for m in range(2)]
                for m in range(2):
                    rqp = ppt.tile([128, 1], F32, tag="tiny", name="tiny", bufs=1)
                    nc.tensor.matmul(rqp[:], rrow[0:1, m * 128:(m + 1) * 128],
                                     ones_c[0:1, 0:1])
                    nc.vector.tensor_copy(rq_col[m][:], rqp[:])
                # rk broadcast to 128 partitions
                rkb = psm.tile([128, 256], BF16, tag="rkb", name="rkb")
                rkbp = pbc.tile([128, 256], F32, tag="bc", name="rkbp")
                nc.tensor.matmul(rkbp[:], ones_r[:], rrow[0:1, 256:512])
                nc.vector.tensor_copy(rkb[:], rkbp[:])

                # ---- scores + softmax + transposed block-diag A ----
                bdT = [psm.tile([128, 128], BF16, tag=f"bdT{m}", name=f"bdT{m}")
                       for m in range(2)]
                invr = [psm.tile([128, 1], F32, tag=f"invr{m}", name=f"invr{m}")
                        for m in range(2)]
                for m in range(2):
                    s_ps = pps.tile([128, 256], F32, tag="s", name="s")
                    for nt in range(NT):
                        nc.tensor.matmul(
                            s_ps[:],
                            qk_sb[:, nt * 512 + m * 128: nt * 512 + m * 128 + 128],
                            qk_sb[:, nt * 512 + 256: nt * 512 + 512],
                            start=(nt == 0), stop=(nt == NT - 1))
                    a_sb = psm.tile([128, 32], F32, tag=f"a{m}", name=f"a{m}")
                    for j in range(4):
                        h = m * 4 + j
                        nc.vector.scalar_tensor_tensor(
                            a_sb[32 * j:32 * j + 32, :],
                            s_ps[32 * j:32 * j + 32, 32 * h:32 * h + 32],
                            rq_col[m][32 * j:32 * j + 32, :],
                            rkb[32 * j:32 * j + 32, 32 * h:32 * h + 32],
                            ALU.mult, ALU.mult)
                    e_sb = psm.tile([128, 32], BF16, tag=f"e{m}", name=f"e{m}")
                    rs = psm.tile([128, 1], F32, tag=f"rs{m}", name=f"rs{m}")
                    nc.scalar.activation(e_sb[:], a_sb[:], AF.Exp,
                                         accum_out=rs[:])
                    nc.vector.reciprocal(invr[m][:], rs[:])
                    eT = psm.tile([128, 32], BF16, tag=f"eT{m}", name=f"eT{m}")
                    nc.vector.transpose(eT[:], e_sb[:])
                    nc.vector.memset(bdT[m][:], 0.0)
                    for j in range(4):
                        nc.vector.tensor_copy(
                            bdT[m][32 * j:32 * j + 32, 32 * j:32 * j + 32],
                            eT[32 * j:32 * j + 32, :])

                # ---- attention @ v (block-diag), normalize, GAP ----
                attn = [pmid.tile([128, N], BF16, tag=f"at{m}", name=f"at{m}")
                        for m in range(2)]
                for m in range(2):
                    for fp in range(4):
                        a_ps = ppb.tile([128, 1024], F32, tag="ps", name="ps")
                        for half in range(2):
                            f = fp * 2 + half
                            rhs = vp3[m][:, f * 8 + 1: f * 8 + 9, 1:65]
                            nc.tensor.matmul(
                                a_ps[:, half * 512:(half + 1) * 512],
                                bdT[m][:], rhs)
                        dst = attn[m][:, fp * 1024:(fp + 1) * 1024]
                        if fp % 2 == 0:
                            nc.scalar.activation(dst, a_ps[:], AF.Copy,
                                                 scale=invr[m][:])
                        else:
                            nc.vector.tensor_scalar(dst, a_ps[:], invr[m][:],
                                                    None, ALU.mult)

                # ---- channel gate from GAP = invr * (A_exp @ rowsum(v)) ----
                gap_bf = [psm.tile([128, 1], BF16, tag=f"gb{m}", name=f"gb{m}")
                          for m in range(2)]
                for m in range(2):
                    vs_bf = psm.tile([128, 1], BF16, tag=f"vsb{m}",
                                     name=f"vsb{m}")
                    vs_f = psm.tile([128, 1], F32, tag=f"vsf{m}", name=f"vsf{m}")
                    nc.vector.reduce_sum(vs_f[:], vsump[m][:],
                                         axis=mybir.AxisListType.X)
                    nc.vector.tensor_copy(vs_bf[:], vs_f[:])
                    g_ps = ppt.tile([128, 1], F32, tag="tiny", name="gps",
                                    bufs=1)
                    nc.tensor.matmul(g_ps[:], bdT[m][:], vs_bf[:])
                    nc.vector.tensor_scalar(gap_bf[m][:], g_ps[:],
                                            invr[m][:], None, ALU.mult)
                cm1_ps = ppt.tile([32, 1], F32, tag="tiny", name="cm1", bufs=1)
                for m in range(2):
                    nc.tensor.matmul(cm1_ps[:], ciw1[m][:], gap_bf[m][:],
                                     start=(m == 0), stop=(m == 1))
                cm1g = psm.tile([32, 1], BF16, tag="cm1g", name="cm1g")
                nc.scalar.activation(cm1g[:], cm1_ps[:], AF.Gelu, bias=tci[:])
                cmsig = [psm.tile([128, 1], F32, tag=f"cs{m}", name=f"cs{m}")
                         for m in range(2)]
                for m in range(2):
                    cm2_ps = ppt.tile([128, 1], F32, tag="tiny", name="cm2",
                                      bufs=1)
                    nc.tensor.matmul(cm2_ps[:], ciw2[:, m * 128:(m + 1) * 128],
                                     cm1g[:])
                    cmt = psm.tile([128, 1], F32, tag=f"ct{m}", name=f"ct{m}")
                    nc.scalar.activation(cmt[:], cm2_ps[:], AF.Tanh, scale=0.5,
                                         bias=cib2[m][:])
                    nc.vector.tensor_scalar(cmsig[m][:], cmt[:], 0.5, 0.5,
                                            ALU.mult, ALU.add)

                # ---- depthwise conv + spatial gate, interleaved per pair ----
                convb = [pmid.tile([128, N], BF16, tag=f"cb{m}", name=f"cb{m}")
                         for m in range(2)]
                dvp = [pmid.tile([128, N], BF16, tag=f"dvp{m}", name=f"dvp{m}")
                       for m in range(2)]
                th = pmid.tile([128, N], BF16, tag="th", name="th")
                sigb = th
                for fp in range(4):
                    for m in range(2):
                        c_ps = ppb.tile([128, 1024], F32, tag="ps", name="ps")
                        for half in range(2):
                            f = fp * 2 + half
                            sl = c_ps[:, half * 512:(half + 1) * 512]
                            for i, t in enumerate(PE_TAPS):
                                dy, dx = TAPS[t]
                                rhs = vp3[m][:, f * 8 + 1 + dy: f * 8 + 9 + dy,
                                             1 + dx: 65 + dx]
                                nc.tensor.matmul(
                                    sl, dwdg[m][:, t * 128:(t + 1) * 128], rhs,
                                    start=(i == 0), stop=(i == len(PE_TAPS) - 1))
                        nc.scalar.activation(
                            convb[m][:, fp * 1024:(fp + 1) * 1024], c_ps[:],
                            AF.Gelu, bias=tdw[m][:], scale=sdw[m][:])
                    for half in range(2):
                        f = fp * 2 + half
                        sm1_ps = ppt.tile([16, 512], F32, tag="tiny",
                                          name="sm1", bufs=1)
                        for m in range(2):
                            nc.tensor.matmul(
                                sm1_ps[:], siw1[m][:],
                                convb[m][:, f * 512:(f + 1) * 512],
                                start=(m == 0), stop=(m == 1))
                        sm1g = psm.tile([16, 512], BF16, tag="sm1g",
                                        name="sm1g")
                        nc.scalar.activation(sm1g[:], sm1_ps[:], AF.Gelu,
                                             bias=tsi[:])
                        bc_ps = pbc.tile([128, 512], F32, tag="bc", name="bc")
                        nc.tensor.matmul(bc_ps[:], siw2[:], sm1g[:])
                        nc.scalar.activation(th[:, f * 512:(f + 1) * 512],
                                             bc_ps[:], AF.Tanh, scale=0.5,
                                             bias=sib2v[:])
                # ---- combine + projection + store (streamed by half) ----
                z = dvp
                for fh in range(2):
                    for q in range(2):
                        qsl = slice(fh * 2048 + q * 1024,
                                    fh * 2048 + (q + 1) * 1024)
                        nc.vector.tensor_scalar(sigb[:, qsl], th[:, qsl],
                                                0.5, 0.5, ALU.mult, ALU.add)
                        for m in range(2):
                            nc.vector.tensor_mul(attn[m][:, qsl],
                                                 attn[m][:, qsl], sigb[:, qsl])
                            nc.vector.scalar_tensor_tensor(
                                z[m][:, qsl], convb[m][:, qsl], cmsig[m][:],
                                attn[m][:, qsl], ALU.mult, ALU.add)
                    for m2 in range(2):
                        ost = pout.tile([128, 2048], F32, tag="ost", name="ost")
                        for fp in range(2):
                            o_ps = ppb.tile([128, 1024], F32, tag="ps",
                                            name="ps")
                            for half in range(2):
                                f = fh * 4 + fp * 2 + half
                                for kk in range(2):
                                    nc.tensor.matmul(
                                        o_ps[:, half * 512:(half + 1) * 512],
                                        wproj[kk][:, m2 * 128:(m2 + 1) * 128],
                                        z[kk][:, f * 512:(f + 1) * 512],
                                        start=(kk == 0), stop=(kk == 1))
                            dst = ost[:, fp * 1024:(fp + 1) * 1024]
                            nc.scalar.activation(dst, o_ps[:], AF.Identity,
                                                 bias=bproj[m2][:])
                        nc.sync.dma_start(
                            out_d[m2 * 128:(m2 + 1) * 128,
                                  fh * 2048:(fh + 1) * 2048], ost[:])
    nc.finalize()
    return nc


_prog_cache = {}


def _get_program(si_b2_half):
    key = float(si_b2_half)
    if key not in _prog_cache:
        _prog_cache[key] = _build_program(key)
    return _prog_cache[key]


def kernel(**inputs):
    x = np.asarray(inputs["x"], np.float32)           # (8, 256, 64, 64)
    B = x.shape[0]
    consts = _build_consts(inputs)
    si_b2_half = 0.5 * float(np.asarray(inputs["si_b2"]).reshape(-1)[0])
    nc = _get_program(si_b2_half)

    in_maps = []
    for b in range(B):
        m = {"x": np.ascontiguousarray(x[b].reshape(C, N))}
        m.update(consts)
        in_maps.append(m)
    res = run_bass_kernel_spmd(nc, in_maps, list(range(NCORES)))
    out = np.stack([res.results[i]["out"] for i in range(B)])
    return out.reshape(B, C, HW_SIDE, HW_SIDE).astype(np.float32)


if __name__ == "__main__":
    # self-test only; the grader imports kernel() and never runs this block
    sys.path.insert(0, "/root/problem")
    import reference
    ins = {k: np.asarray(v) for k, v in reference.setup_inputs().items()}
    got = kernel(**ins)
    want = np.asarray(reference.reference(**ins))
    err = np.abs(got - want).max() / (np.abs(want).max() + 1e-12)
    print("abs-rel err:", err)

